# revision 9
# baseline (speedup 1.0000x reference)
"""Chamfer loss kernel for Trainium2 (8 NeuronCores).

Problem: array1 [4, 8192, 3], array2 [4, 8192, 3] (f32).
loss = (mean(sqrt(min_m d(n,m))) + mean(sqrt(min_n d(n,m)))) / 2
with d = squared euclidean distance, clamped at 0.

Sharding: core c -> (batch b = c//2, N-half h = c%2). Each core gets
4096 rows of array1[b] and all 8192 rows of array2[b], computes the
[4096, 8192] distance block via the PE (K=4 matmul trick:
P = -2*a.b + |b|^2), then q = relu(P + |a|^2) on the scalar engine
(cast to bf16), and row-min (dist1) plus a running column-min (dist2
partial) on the vector engine. Host combines: per-batch column mins
across the two half cores, then sqrt + mean in f64.
"""

import os
import sys
import numpy as np

for p in ("/opt/trn_rl_repo",):
    if p not in sys.path:
        sys.path.insert(0, p)

B, N, M = 4, 8192, 8192
N_CORES = 8
N_LOC = N // 2          # rows per core
RT_N = N_LOC // 128     # 32 row tiles
MM_N = 512              # matmul free dim (f32 max)
PSUM_COLS = 2048        # 4 banks per psum tile
G_N = M // PSUM_COLS    # 4 psum groups per row tile

QDT_NAME = os.environ.get("KERNEL_QDT", "bfloat16")  # bfloat16 | float32
ROWMIN_MODE = os.environ.get("KERNEL_ROWMIN", "reduce")  # reduce | ttr | tree
# 128 = no on-device partition fold (DVE tensor_tensor requires equal base
# partitions for both SBUF inputs, so cross-partition folds are illegal)
FOLD_TO = int(os.environ.get("KERNEL_FOLD_TO", "128"))

_CACHE = {}


def _build_nc():
    import concourse.mybir as mybir
    from concourse import bacc
    from concourse.tile import TileContext

    f32 = mybir.dt.float32
    qdt = getattr(mybir.dt, QDT_NAME)

    # Bacc (not raw Bass): its compile() runs move_matmul_waits_to_ldweights
    # + generate_event_semaphores, which split Tile's multi-wait instructions
    # down to the 1-wait-per-instruction limit the TRN2 encodings allow.
    nc = bacc.Bacc()

    # ahat+bhat packed in one tensor -> one DMA -> one HWDGE semaphore
    # (separate DMAs land on different HW queues and the lowered LDW
    # instruction only supports a single sync wait)
    abhat_d = nc.declare_dram_parameter("abhat", [4, M + N_LOC], f32, isOutput=False)
    asq_d = nc.declare_dram_parameter("asq", [128, RT_N], f32, isOutput=False)
    rowmin_d = nc.declare_dram_parameter("rowmin", [128, RT_N], f32, isOutput=True)
    colmin_d = nc.declare_dram_parameter("colmin", [FOLD_TO, M], qdt, isOutput=True)

    with TileContext(nc) as tc:
        with (
            tc.tile_pool(name="const", bufs=1) as cpool,
            tc.tile_pool(name="psum", bufs=2, space="PSUM") as ppool,
            tc.tile_pool(name="qrow", bufs=2) as qpool,
            tc.tile_pool(name="acc", bufs=1) as apool,
        ):
            abhat = cpool.tile([4, M + N_LOC], f32)
            asq = cpool.tile([128, RT_N], f32)
            nc.sync.dma_start(out=abhat[:, :], in_=abhat_d[:, :])
            nc.sync.dma_start(out=asq[:, :], in_=asq_d[:, :])
            bhat = abhat[:, 0:M]
            ahat = abhat[:, M:M + N_LOC]

            # Pre-touch asq on ScalarE: the activation instruction encoding
            # only fits ONE sync wait, and the real activations must wait on
            # PE. This copy makes ScalarE observe the asq DMA semaphore once.
            pretouch = cpool.tile([128, 1], f32)
            nc.scalar.copy(pretouch[:, :], asq[:, 0:1])

            colacc = apool.tile([128, M], qdt)
            rowmin = apool.tile([128, RT_N], f32)
            scratch = apool.tile([128, M // 2], qdt) if ROWMIN_MODE == "tree" else None

            for rt in range(RT_N):
                qrow = qpool.tile([128, M], qdt)
                for g in range(G_N):
                    psum = ppool.tile([128, PSUM_COLS], f32)
                    for j in range(PSUM_COLS // MM_N):
                        ct = g * (PSUM_COLS // MM_N) + j
                        nc.tensor.matmul(
                            psum[:, j * MM_N:(j + 1) * MM_N],
                            ahat[:, rt * 128:(rt + 1) * 128],
                            bhat[:, ct * MM_N:(ct + 1) * MM_N],
                            start=True,
                            stop=True,
                        )
                    nc.scalar.activation(
                        qrow[:, g * PSUM_COLS:(g + 1) * PSUM_COLS],
                        psum[:, :],
                        mybir.ActivationFunctionType.Relu,
                        bias=asq[:, rt:rt + 1],
                        scale=1.0,
                    )

                # dist1: row-wise min of this row tile
                if ROWMIN_MODE == "reduce":
                    nc.vector.tensor_reduce(
                        out=rowmin[:, rt:rt + 1],
                        in_=qrow[:, :],
                        axis=mybir.AxisListType.X,
                        op=mybir.AluOpType.min,
                    )
                elif ROWMIN_MODE == "ttr":
                    nc.vector.tensor_tensor_reduce(
                        out=qrow[:, :],
                        in0=qrow[:, :],
                        in1=qrow[:, :],
                        scale=1.0,
                        scalar=3.0e38,
                        op0=mybir.AluOpType.min,
                        op1=mybir.AluOpType.min,
                        accum_out=rowmin[:, rt:rt + 1],
                    )
                else:  # tree
                    half = M // 2
                    nc.vector.tensor_tensor(
                        scratch[:, :half], qrow[:, :half], qrow[:, half:],
                        mybir.AluOpType.min,
                    )
                    while half > 8:
                        h2 = half // 2
                        nc.vector.tensor_tensor(
                            scratch[:, :h2], scratch[:, :h2], scratch[:, h2:half],
                            mybir.AluOpType.min,
                        )
                        half = h2
                    nc.vector.tensor_reduce(
                        out=rowmin[:, rt:rt + 1],
                        in_=scratch[:, :half],
                        axis=mybir.AxisListType.X,
                        op=mybir.AluOpType.min,
                    )

                # dist2 partial: running column min across row tiles
                if rt == 0:
                    nc.vector.tensor_copy(colacc[:, :], qrow[:, :])
                else:
                    nc.vector.tensor_tensor(
                        colacc[:, :], colacc[:, :], qrow[:, :],
                        mybir.AluOpType.min,
                    )

            # fold colacc partitions 128 -> FOLD_TO
            f = 64
            while f >= FOLD_TO:
                nc.vector.tensor_tensor(
                    colacc[0:f, :], colacc[0:f, :], colacc[f:2 * f, :],
                    mybir.AluOpType.min,
                )
                f //= 2

            nc.sync.dma_start(out=rowmin_d[:, :], in_=rowmin[:, :])
            nc.sync.dma_start(out=colmin_d[:, :], in_=colacc[0:FOLD_TO, :])

    nc.compile()
    return nc


def _get_nc():
    key = (QDT_NAME, ROWMIN_MODE, FOLD_TO)
    if key not in _CACHE:
        _CACHE[key] = _build_nc()
    return _CACHE[key]


def _prep_core_inputs(array1, array2, c):
    b, h = c // 2, c % 2
    a = np.asarray(array1[b, h * N_LOC:(h + 1) * N_LOC, :], dtype=np.float32)
    bb = np.asarray(array2[b], dtype=np.float32)
    ahat = np.ascontiguousarray(
        np.stack([-2.0 * a[:, 0], -2.0 * a[:, 1], -2.0 * a[:, 2],
                  np.ones(N_LOC, np.float32)])).astype(np.float32)
    bhat = np.ascontiguousarray(
        np.stack([bb[:, 0], bb[:, 1], bb[:, 2],
                  (bb * bb).sum(-1)])).astype(np.float32)
    asq = np.ascontiguousarray((a * a).sum(-1).reshape(RT_N, 128).T).astype(np.float32)
    abhat = np.ascontiguousarray(np.concatenate([bhat, ahat], axis=1))
    return {"abhat": abhat, "asq": asq}


def run_on_hw(array1, array2, trace=False):
    from concourse import bass_utils

    nc = _get_nc()
    in_maps = [_prep_core_inputs(array1, array2, c) for c in range(N_CORES)]
    res = bass_utils.run_bass_kernel_spmd(
        nc, in_maps, core_ids=list(range(N_CORES)), trace=trace,
    )
    return res


def finish_on_host(results):
    dist1 = np.empty((B, N), np.float64)
    dist2 = np.empty((B, M), np.float64)
    for b in range(B):
        parts = []
        for h in range(2):
            r = results[2 * b + h]
            rm = np.asarray(r["rowmin"], dtype=np.float64)  # [128, RT_N]
            dist1[b, h * N_LOC:(h + 1) * N_LOC] = rm.T.reshape(N_LOC)
            parts.append(np.asarray(r["colmin"], dtype=np.float64).min(axis=0))
        dist2[b] = np.minimum(parts[0], parts[1])
    loss = 0.5 * (np.mean(np.sqrt(dist1)) + np.mean(np.sqrt(dist2)))
    return np.float32(loss)


def kernel(array1, array2):
    res = run_on_hw(array1, array2, trace=False)
    return finish_on_host(res.results)
